# revision 38
# baseline (speedup 1.0000x reference)
"""Multi-head attention (B=2, N=4096, C=512, H=8) on 8 trn2 NeuronCores.

Sharding: core -> (batch b = core//4, head-pair hp = core%4): data parallel
over B, tensor parallel over heads (2 per core), column-sharded qkv weights,
row-sharded proj weights. Each core returns TWO unnormalized per-head proj
partials (bf16) plus the softmax denominators (f32); the host divides by the
denominators, sums the 4 head-pairs per batch, and adds proj_b.

Per-core device kernel (flash-style, nothing N^2 ever hits HBM):
  qT/kT  [128(=2 heads x 64 feat), 4096]  <- wqk^T @ x^T   (bf16 matmuls)
  v_sb   [128 keys, 32 m-tiles, 2x65]     <- x^T^T @ wv (+bias), ones cols
  items (key m-tile, head) alternate heads so the K=64 score matmuls land
  on alternating 64x128 PE row-tiles (T0 for h0: SBUF partitions 0-63, T8
  for h1: 64-127) and run pairwise-concurrently. Chunks of 3 items share
  one [128, 1536] PSUM tile; each item owns a 512-col bank, so concurrent
  row-tiles never write the same bank.
    S^T chunk in PSUM  <- kT-x-qT   (row-tiled score matmuls)
    E = exp(SCALE*S^T): most chunks via one ACTIVATE on ACT; every
      SCH_PERIOD-th chunk on DVE(+Pool) via a Schraudolph bf16 exp:
      bits = int16(s*SCALE*log2e*128 + (127-sigma)*128), bitcast as bf16
      (tensor_scalar affine psum->f32 sbuf, then tensor_copy cast ->int16)
    out^T [65, 512] PSUM += v_aug-x-E  (row 64 = denominator via ones col)
  per g (512 queries), after both heads' AV accumulations:
    drain av psums -> asb2 [128, 512] bf16 (h0 rows 0-63, h1 rows 64-127)
    + den [2, 512] f32; proj as row-tiled 64x128 pairs (T0: h0, T8: h1)
    into two banks of one psum slot; drain to bf16, DMA out unnormalized.
"""

import numpy as np

_state = {}

B, N, C, H, DH = 2, 4096, 512, 8, 64
SCALE = DH ** -0.5
GQ = 512          # queries per group
NG = N // GQ      # 8 groups
MT = N // 128     # 32 key m-tiles
LOG2E = 1.4426950408889634
SIGMA = 0.0579
SCH_A = float(SCALE * LOG2E * 128.0)          # schraudolph affine mult
SCH_B = float((127.0 - SIGMA) * 128.0)        # schraudolph affine add
SCH_PERIOD = 2    # every SCH_PERIOD-th chunk uses the DVE/Pool exp (0=off)
POOL_CAST = True  # cast f32->i16 on gpsimd (False: on DVE)
SKEW = 4


def _build_nc(sch_period=None, pool_cast=None):
    from contextlib import ExitStack
    from collections import deque

    import concourse.bacc as bacc
    import concourse.tile as tile
    from concourse import mybir

    if sch_period is None:
        sch_period = SCH_PERIOD
    if pool_cast is None:
        pool_cast = POOL_CAST

    bf16 = mybir.dt.bfloat16
    f32 = mybir.dt.float32
    i16 = mybir.dt.int16
    EXP = mybir.ActivationFunctionType.Exp
    MULT = mybir.AluOpType.mult
    ADD = mybir.AluOpType.add

    nc = bacc.Bacc(None, target_bir_lowering=False)
    with tile.TileContext(nc) as tc, ExitStack() as ctx:
        dram = ctx.enter_context(tc.tile_pool(name="dram", bufs=1, space="DRAM"))
        xt_d = dram.tile([C, N], bf16, kind="ExternalInput", name="xt",
                         uniquify=False, tag="dxt")
        wqk_d = dram.tile([C, 256], bf16, kind="ExternalInput", name="wqk",
                          uniquify=False, tag="dwqk")
        bqk_d = dram.tile([128, 2], f32, kind="ExternalInput", name="bqk",
                          uniquify=False, tag="dbqk")
        wv_d = dram.tile([C, 128], bf16, kind="ExternalInput", name="wv",
                         uniquify=False, tag="dwv")
        bv_d = dram.tile([128, 128], bf16, kind="ExternalInput", name="bv",
                         uniquify=False, tag="dbv")
        pw_d = dram.tile([128, 512], bf16, kind="ExternalInput", name="pw",
                         uniquify=False, tag="dpw")
        out0_d = dram.tile([N, C], bf16, kind="ExternalOutput", name="out0",
                           uniquify=False, tag="dout0")
        out1_d = dram.tile([N, C], bf16, kind="ExternalOutput", name="out1",
                           uniquify=False, tag="dout1")
        den_d = dram.tile([NG, 2 * GQ], f32, kind="ExternalOutput", name="den",
                          uniquify=False, tag="dden")

        const = ctx.enter_context(tc.tile_pool(name="const", bufs=1))
        wqk_sb = const.tile([128, 4, 256], bf16, name="wqk_sb", tag="wqk_sb")
        nc.gpsimd.dma_start(wqk_sb[:], wqk_d.rearrange("(k p) f -> p k f", p=128))
        wv_sb = const.tile([128, 4, 128], bf16, name="wv_sb", tag="wv_sb")
        nc.gpsimd.dma_start(wv_sb[:], wv_d.rearrange("(k p) f -> p k f", p=128))
        bqk_sb = const.tile([128, 2], f32, name="bqk_sb", tag="bqk_sb")
        nc.gpsimd.dma_start(bqk_sb[:], bqk_d[:])
        bv_sb = const.tile([128, 128], bf16, name="bv_sb", tag="bv_sb")
        nc.gpsimd.dma_start(bv_sb[:], bv_d[:])
        pw_sb = const.tile([128, 512], bf16, name="pw_sb", tag="pw_sb")
        nc.gpsimd.dma_start(pw_sb[:], pw_d[:])

        persist = ctx.enter_context(tc.tile_pool(name="persist", bufs=1))
        qT = persist.tile([128, N], bf16, name="qT", tag="qT")
        kT = persist.tile([128, N], bf16, name="kT", tag="kT")
        vsb = persist.tile([128, MT, 130], bf16, name="vsb", tag="vsb")
        vones = vsb.rearrange("p m (a b) -> p m a b", a=2)
        nc.vector.memset(vones[:, :, 0, 64:65], 1.0)
        nc.vector.memset(vones[:, :, 1, 64:65], 1.0)

        xpool = ctx.enter_context(tc.tile_pool(name="xp", bufs=4))
        spool = ctx.enter_context(tc.tile_pool(name="sp", bufs=2, space="PSUM"))
        apool = ctx.enter_context(tc.tile_pool(name="ap", bufs=2, space="PSUM"))
        # pp tiles reuse the av banks (apool) right after the av drains
        epool = ctx.enter_context(tc.tile_pool(name="ep", bufs=7))
        tpool = ctx.enter_context(tc.tile_pool(name="tp", bufs=2))
        opool = ctx.enter_context(tc.tile_pool(name="op", bufs=5))
        mpool = ctx.enter_context(tc.tile_pool(name="mp", bufs=2))
        dpool = ctx.enter_context(tc.tile_pool(name="dp", bufs=2))

        xt_r = xt_d.rearrange("(k p) n -> p k n", p=128)

        next_qk = [0]
        next_v = [0]
        xtiles = {}

        def emit_qk(g):
            xtile = xpool.tile([128, 4, GQ], bf16, name="xtile", tag="xtile")
            xtiles[g] = xtile
            for k in range(4):
                nc.sync.dma_start(xtile[:, k, :],
                                  xt_r[:, k, GQ * g:GQ * (g + 1)])
            qkp = spool.tile([128, 2 * GQ], f32, name="qkp", tag="sch")
            for k in range(4):
                nc.tensor.matmul(qkp[:, 0:512], wqk_sb[:, k, 0:128],
                                 xtile[:, k, :], start=(k == 0), stop=(k == 3))
            for k in range(4):
                nc.tensor.matmul(qkp[:, 512:1024], wqk_sb[:, k, 128:256],
                                 xtile[:, k, :], start=(k == 0), stop=(k == 3))
            nc.vector.tensor_scalar_add(qT[:, GQ * g:GQ * (g + 1)],
                                        qkp[:, 0:512], bqk_sb[:, 0:1])
            nc.vector.tensor_scalar_add(kT[:, GQ * g:GQ * (g + 1)],
                                        qkp[:, 512:1024], bqk_sb[:, 1:2])

        def emit_v4(vg):
            # all 4 key m-tiles of x-group vg in one psum slot (cols 128t)
            xtile = xtiles[vg]
            vp = spool.tile([128, 2 * GQ], f32, name="vp", tag="sch")
            for t in range(4):
                for k in range(4):
                    nc.tensor.matmul(vp[:, 128 * t:128 * (t + 1)],
                                     xtile[:, k, 128 * t:128 * (t + 1)],
                                     wv_sb[:, k, :],
                                     start=(k == 0), stop=(k == 3))
            bvv = bv_sb.rearrange("p (a b) -> p a b", a=2)
            for t in range(4):
                m = 4 * vg + t
                src = vp[:, 128 * t:128 * (t + 1)].rearrange(
                    "p (a b) -> p a b", a=2)
                dst = vsb[:, m, :].rearrange("p (a b) -> p a b", a=2)
                nc.vector.tensor_add(dst[:, :, 0:64], src, bvv)

        def need_qk(g):
            while next_qk[0] <= g:
                emit_qk(next_qk[0])
                next_qk[0] += 1

        def need_v(m):
            while next_v[0] <= m:
                vg = next_v[0] // 4
                need_qk(vg)
                emit_v4(vg)
                next_v[0] += 4

        # chunk list: per query group g, items (key m-tile, head) with heads
        # alternating; chunk sizes [3]*20 + [2, 2] -> 22 chunks per group.
        chunk_list = []
        for g in range(NG):
            items = []
            for m in range(MT):
                items.append((m, 0))
                items.append((m, 1))
            pos = 0
            for s in [3] * 20 + [2, 2]:
                chunk_list.append((g, items[pos:pos + s]))
                pos += s
        last_chunk_of_g = {}
        for idx, (g, ch) in enumerate(chunk_list):
            last_chunk_of_g[g] = idx

        av_tiles = {}

        def emit_scores(g, ch, use_sch):
            need_qk(max(g, ch[-1][0] // 4))
            w = 512 * len(ch)
            st = spool.tile([128, 3 * GQ], f32, name="st", tag="sch")
            for j, (m, h) in enumerate(ch):
                nc.tensor.matmul(st[:, 512 * j:512 * (j + 1)],
                                 kT[64 * h:64 * h + 64, 128 * m:128 * (m + 1)],
                                 qT[64 * h:64 * h + 64, GQ * g:GQ * (g + 1)],
                                 start=True, stop=True)
            if not use_sch:
                et = epool.tile([128, 3 * GQ], bf16, name="et", tag="et")
                nc.scalar.activation(et[:, 0:w], st[:, 0:w], EXP, scale=SCALE)
                return et, False
            eti = epool.tile([128, 3 * GQ], i16, name="eti", tag="et")
            nc.vector.tensor_scalar(eti[:, 0:w], st[:, 0:w], SCH_A, SCH_B,
                                    MULT, ADD)
            return eti, True

        def emit_av(g, ch, et, is_i16):
            need_v(ch[-1][0])
            for j, (m, h) in enumerate(ch):
                if (g, h) not in av_tiles:
                    av_tiles[(g, h)] = apool.tile([128, 512], f32,
                                                  name="avt", tag="av")
                a = av_tiles[(g, h)]
                sl = et[:, 512 * j:512 * (j + 1)]
                if is_i16:
                    sl = sl.bitcast(bf16)
                nc.tensor.matmul(a[0:65, :], vsb[:, m, 65 * h:65 * h + 65],
                                 sl, start=(m == 0), stop=(m == MT - 1),
                                 skip_group_check=True)

        def emit_post(g):
            a0 = av_tiles.pop((g, 0))
            a1 = av_tiles.pop((g, 1))
            asb2 = opool.tile([128, 512], bf16, name="asb2", tag="asb2")
            nc.vector.tensor_copy(asb2[0:64, :], a0[0:64, :])
            nc.vector.tensor_copy(asb2[64:128, :], a1[0:64, :])
            den = dpool.tile([1, 2 * GQ], f32, name="den", tag="den")
            nc.vector.tensor_copy(den[0:1, 0:512], a0[64:65, :])
            nc.vector.tensor_copy(den[0:1, 512:1024], a1[64:65, :])
            nc.sync.dma_start(den_d[g:g+1, :], den[0:1, :])
            for t in range(4):
                pp = spool.tile([128, 2 * GQ], f32, name="pp", tag="sch")
                nc.tensor.matmul(pp[:, 0:512],
                                 asb2[0:64, 128 * t:128 * (t + 1)],
                                 pw_sb[0:64, :], start=True, stop=True)
                nc.tensor.matmul(pp[:, 512:1024],
                                 asb2[64:128, 128 * t:128 * (t + 1)],
                                 pw_sb[64:128, :], start=True, stop=True)
                t0 = opool.tile([128, 512], bf16, name="t0", tag="t0")
                nc.scalar.copy(t0[:], pp[:, 0:512])
                t1 = opool.tile([128, 512], bf16, name="t1", tag="t1")
                nc.scalar.copy(t1[:], pp[:, 512:1024])
                r0 = GQ * g + 128 * t
                nc.sync.dma_start(out0_d[r0:r0 + 128, :], t0[:])
                nc.sync.dma_start(out1_d[r0:r0 + 128, :], t1[:])

        inflight = deque()
        for idx, (g, ch) in enumerate(chunk_list):
            use_sch = (sch_period > 0 and idx % sch_period == sch_period // 2
                       and idx % 22 != 21)
            et, is_i16 = emit_scores(g, ch, use_sch)
            inflight.append(((idx, g, ch), et, is_i16))
            if len(inflight) > SKEW:
                (fidx, fg, fch), fet, fi16 = inflight.popleft()
                emit_av(fg, fch, fet, fi16)
                if fidx == last_chunk_of_g[fg]:
                    emit_post(fg)
        while inflight:
            (fidx, fg, fch), fet, fi16 = inflight.popleft()
            emit_av(fg, fch, fet, fi16)
            if fidx == last_chunk_of_g[fg]:
                emit_post(fg)

    nc.compile()
    return nc


def _get_nc():
    if "nc" not in _state:
        _state["nc"] = _build_nc()
    return _state["nc"]


def _make_in_maps(x, qkv_w, qkv_b, proj_w):
    import ml_dtypes
    bf = ml_dtypes.bfloat16
    x = np.asarray(x, np.float32)
    qkv_w = np.asarray(qkv_w, np.float32)
    qkv_b = np.asarray(qkv_b, np.float32)
    proj_w = np.asarray(proj_w, np.float32)
    in_maps = []
    for core in range(8):
        b, hp = divmod(core, 4)
        h0, h1 = 2 * hp, 2 * hp + 1
        xt = np.ascontiguousarray(x[b].T).astype(bf)
        rq = np.concatenate([qkv_w[64 * h0:64 * h0 + 64],
                             qkv_w[64 * h1:64 * h1 + 64]], 0)
        rk = np.concatenate([qkv_w[C + 64 * h0:C + 64 * h0 + 64],
                             qkv_w[C + 64 * h1:C + 64 * h1 + 64]], 0)
        wqk = np.ascontiguousarray(np.concatenate([rq, rk], 0).T).astype(bf)
        bq = np.concatenate([qkv_b[64 * h0:64 * h0 + 64],
                             qkv_b[64 * h1:64 * h1 + 64]])
        bk = np.concatenate([qkv_b[C + 64 * h0:C + 64 * h0 + 64],
                             qkv_b[C + 64 * h1:C + 64 * h1 + 64]])
        bqk = np.ascontiguousarray(np.stack([bq, bk], 1)).astype(np.float32)
        rv = np.concatenate([qkv_w[2 * C + 64 * h0:2 * C + 64 * h0 + 64],
                             qkv_w[2 * C + 64 * h1:2 * C + 64 * h1 + 64]], 0)
        wv = np.ascontiguousarray(rv.T).astype(bf)
        bvrow = np.concatenate([qkv_b[2 * C + 64 * h0:2 * C + 64 * h0 + 64],
                                qkv_b[2 * C + 64 * h1:2 * C + 64 * h1 + 64]])
        bv = np.ascontiguousarray(
            np.broadcast_to(bvrow[None, :], (128, 128))).astype(bf)
        pw = np.ascontiguousarray(
            proj_w[:, 128 * hp:128 * hp + 128].T).astype(bf)
        in_maps.append(dict(xt=xt, wqk=wqk, bqk=bqk, wv=wv, bv=bv, pw=pw))
    return in_maps


def _gather(results, proj_b):
    proj_b = np.asarray(proj_b, np.float32)
    out = np.empty((B, N, C), np.float32)
    for b in range(B):
        acc = None
        for hp in range(4):
            r = results[4 * b + hp]
            dd = np.asarray(r["den"], np.float32).reshape(NG, 2, GQ)
            den0 = dd[:, 0, :].reshape(N)
            den1 = dd[:, 1, :].reshape(N)
            p0 = np.asarray(r["out0"], np.float32)
            p1 = np.asarray(r["out1"], np.float32)
            term = p0 / den0[:, None] + p1 / den1[:, None]
            acc = term if acc is None else acc + term
        out[b] = acc + proj_b[None, :]
    return out


def _run(x, qkv_w, qkv_b, proj_w, proj_b, trace=False, tmpdir=None):
    from concourse import bass_utils
    nc = _get_nc()
    in_maps = _make_in_maps(x, qkv_w, qkv_b, proj_w)
    res = bass_utils.run_bass_kernel_spmd(
        nc, in_maps, core_ids=list(range(8)), trace=trace, tmpdir=tmpdir)
    return _gather(res.results, proj_b), res


def kernel(x, qkv_w, qkv_b, proj_w, proj_b):
    out, _ = _run(x, qkv_w, qkv_b, proj_w, proj_b, trace=False)
    return out
